# revision 7
# baseline (speedup 1.0000x reference)
"""Adagnn-with-weight GNN message-passing kernel for 8 Trainium2 NeuronCores.

Reference computation (N=100000 nodes, E=3200000 edges, F=256):
    e1  = segment_sum(edge_val[:,None] * x[edge_col], edge_row)   # spmm
    out = (x - e1 * (learnable_diag + 1)) @ weight + bias

Design notes (evidence from perfetto traces):
  - The wall is SWDGE descriptor generation for the per-edge gather
    (~2 ns/slot, serialized on the gpsimd engine).  Therefore: (a) host-side
    row->tile rebalancing minimizes pad slots (12.5% -> ~5.5%), (b) gathers
    are merged across a super-tile of ST dest tiles (fewer calls -> less
    fixed overhead), (c) nothing else may exceed ~900us.
  - Dest rows are assigned to (core, tile) by a greedy bin-packing that
    equalizes per-(tile, source-block) edge counts across cores (the chunk
    table is shared by all 8 cores).  Output rows are un-permuted on host.
  - DVE one-hot A build pays ~69ns per 128-elem AP row; the output/in0 APs
    are flattened to 2D to (attempt to) amortize it.
  - Epilogue in transposed space: e4T = e1T*(-dscaleT) + xoT fused on DVE,
    bias added via a contract-1 matmul, PSUM->SBUF copies on ACT.
"""

import numpy as np

import concourse.bacc as bacc
import concourse.mybir as mybir
import concourse.tile as tile
from concourse.bass_utils import run_bass_kernel_spmd

FP = mybir.dt.float32
BF = mybir.dt.bfloat16
BF_NP = mybir.dt.np(BF)


class Cfg:
    def __init__(self, n_nodes=100000, n_edges=3200000, f=256, n_cores=8,
                 nb=4, st=4, gather_bufs=7, amat_bufs=2, flat_a=True,
                 rebalance=True):
        self.N = n_nodes
        self.E = n_edges
        self.F = f
        self.NC = n_cores
        self.NB = nb
        self.ST = st
        self.RPC = n_nodes // n_cores
        self.TILES = (self.RPC + 127) // 128
        self.PAD_ROWS = self.TILES * 128
        self.NST = (self.TILES + st - 1) // st
        self.BLK = n_nodes // nb
        assert self.BLK < (1 << 15)
        self.gather_bufs = gather_bufs
        self.amat_bufs = amat_bufs
        self.flat_a = flat_a
        self.rebalance = rebalance


def _assign_rows(cfg, edge_row, edge_col):
    """Greedy bin-packing of dest rows into (core, tile) bins, minimizing
    per-(tile, block) overflow above 8 chunks.  Returns row_perm[N] giving
    the device row ordering: device row (c, t, d) holds original row
    row_perm[c*PAD_ROWS + t*128 + d] (or -1 for unused pad slots)."""
    N, NB, NC, TILES = cfg.N, cfg.NB, cfg.NC, cfg.TILES
    f = np.zeros((N, NB), dtype=np.int64)
    np.add.at(f, (edge_row, edge_col // cfg.BLK), 1)
    if not cfg.rebalance:
        gbin = np.zeros(N, dtype=np.int64)
        for c in range(NC):
            rr = np.arange(c * cfg.RPC, (c + 1) * cfg.RPC)
            gbin[rr] = c * TILES + np.minimum((rr - c * cfg.RPC) // 128,
                                              TILES - 1)
    else:
        tot = f.sum(1)
        nbins = NC * TILES
        capm = np.full((NC, TILES), 128)
        capm[:, -1] = cfg.RPC - (TILES - 1) * 128
        cap = capm.reshape(-1)
        load = np.zeros((nbins, NB), dtype=np.float64)
        cnt = np.zeros(nbins, dtype=np.int64)
        gbin = np.empty(N, dtype=np.int64)
        for r in np.argsort(-tot, kind='stable'):
            nl = load + f[r]
            scores = (np.maximum(nl - 1024.0, 0).sum(axis=1)
                      + nl.max(axis=1) * 1e-3 + (cnt >= cap) * 1e9)
            bb = int(np.argmin(scores))
            gbin[r] = bb
            load[bb] += f[r]
            cnt[bb] += 1

    # row_perm & per-row device slot
    order = np.argsort(gbin, kind='stable')
    row_perm = np.full(NC * cfg.PAD_ROWS, -1, dtype=np.int64)
    row_slot = np.empty(N, dtype=np.int64)  # device row id per original row
    pos_in_bin = np.zeros(NC * TILES, dtype=np.int64)
    gb_sorted = gbin[order]
    # rank within bin
    starts = np.searchsorted(gb_sorted, np.arange(NC * TILES), side='left')
    rank = np.arange(N) - starts[gb_sorted]
    c = gb_sorted // TILES
    t = gb_sorted % TILES
    dev = c * cfg.PAD_ROWS + t * 128 + rank
    row_perm[dev] = order
    row_slot[order] = dev
    return row_perm, row_slot


def _preprocess(cfg, edge_row, edge_col, edge_val, row_slot):
    """Partition + sort + pad the edge list using the device row mapping."""
    edge_row = np.asarray(edge_row).astype(np.int64)
    edge_col = np.asarray(edge_col).astype(np.int64)
    edge_val = np.asarray(edge_val).astype(np.float32)
    NC, TILES, NB, ST, NST, E = cfg.NC, cfg.TILES, cfg.NB, cfg.ST, cfg.NST, cfg.E

    dev = row_slot[edge_row]
    core = dev // cfg.PAD_ROWS
    dloc = dev - core * cfg.PAD_ROWS
    t = dloc >> 7
    d = (dloc & 127).astype(np.float32)
    b = edge_col // cfg.BLK
    cloc = (edge_col - b * cfg.BLK).astype(np.int16)

    # cell order: (super-tile, block, tile-in-st)
    ncell = TILES * NB
    cell_rank = np.zeros((TILES, NB), dtype=np.int64)
    rank = 0
    for stt in range(NST):
        t0, t1 = stt * ST, min((stt + 1) * ST, TILES)
        for bb in range(NB):
            for tt in range(t0, t1):
                cell_rank[tt, bb] = rank
                rank += 1
    assert rank == ncell

    key = core * ncell + cell_rank[t, b]
    order = np.lexsort((cloc, key))
    key_s = key[order]

    counts = np.bincount(key, minlength=NC * ncell).reshape(NC, ncell)
    C = np.ceil(counts.max(axis=0) / 128).astype(np.int64)
    pad_off = np.concatenate([[0], np.cumsum(128 * C)])
    L = int(pad_off[-1])

    starts = np.searchsorted(key_s, np.arange(NC * ncell), side="left")
    rnk = np.arange(E) - starts[key_s]
    pos = (key_s // ncell) * L + pad_off[key_s % ncell] + rnk

    col_pad = np.zeros(NC * L, dtype=np.int16)   # pads gather block row 0
    dest_pad = np.zeros(NC * L, dtype=np.float32)
    val_pad = np.zeros(NC * L, dtype=np.float32)
    col_pad[pos] = cloc[order]
    dest_pad[pos] = d[order]
    val_pad[pos] = edge_val[order]

    col_pad = col_pad.reshape(NC, L)
    dest_pad = dest_pad.reshape(NC, L)
    val_pad = val_pad.reshape(NC, L)

    idx_packed = np.tile(
        col_pad.reshape(NC, L // 16, 16).transpose(0, 2, 1), (1, 8, 1)
    )  # [NC, 128, L//16]

    n_t = np.array([sum(int(C[cell_rank[tt, bb]]) for bb in range(NB))
                    for tt in range(TILES)])
    CT = int(n_t.sum())
    dest_cols = np.zeros((NC, 128, CT), dtype=BF_NP)
    val_cols = np.zeros((NC, 128, CT), dtype=BF_NP)
    tile_coff = np.concatenate([[0], np.cumsum(n_t)])
    for tt in range(TILES):
        toff = int(tile_coff[tt])
        for bb in range(NB):
            r = cell_rank[tt, bb]
            o0, n = int(pad_off[r]), int(C[r])
            if n == 0:
                continue
            seg = slice(o0, o0 + 128 * n)
            dest_cols[:, :, toff:toff + n] = (
                dest_pad[:, seg].reshape(NC, n, 128).transpose(0, 2, 1))
            val_cols[:, :, toff:toff + n] = (
                val_pad[:, seg].reshape(NC, n, 128).transpose(0, 2, 1))
            toff += n

    tables = dict(C=C, cell_rank=cell_rank, pad_off=pad_off, n_t=n_t,
                  tile_coff=tile_coff)
    return tables, idx_packed, dest_cols, val_cols


def _build(cfg, tables):
    F, NB, ST, NST, TILES = cfg.F, cfg.NB, cfg.ST, cfg.NST, cfg.TILES
    C = tables["C"]
    cell_rank = tables["cell_rank"]
    pad_off = tables["pad_off"]
    n_t = tables["n_t"]
    tile_coff = tables["tile_coff"]
    L = int(pad_off[-1])
    C_MAXT = int(n_t.max())
    KC = F // 128

    C_stb = np.zeros((NST, NB), dtype=np.int64)
    gcall_off = np.zeros((NST, NB), dtype=np.int64)
    for stt in range(NST):
        t0, t1 = stt * ST, min((stt + 1) * ST, TILES)
        for bb in range(NB):
            gcall_off[stt, bb] = pad_off[cell_rank[t0, bb]] // 128
            C_stb[stt, bb] = sum(int(C[cell_rank[tt, bb]])
                                 for tt in range(t0, t1))
    C_MAXG = int(C_stb.max())

    nc = bacc.Bacc("TRN2", target_bir_lowering=False, debug=False,
                   num_swdge_queues=4)

    xsrc = nc.dram_tensor("xsrc", [cfg.N, F], BF, kind="ExternalInput")
    xot_d = nc.dram_tensor("xot", [128, TILES, KC, 128], BF,
                           kind="ExternalInput")
    idx_d = nc.dram_tensor("idx", [128, L // 16], mybir.dt.int16,
                           kind="ExternalInput")
    dest_d = nc.dram_tensor("dest", [128, int(n_t.sum())], BF,
                            kind="ExternalInput")
    val_d = nc.dram_tensor("val", [128, int(n_t.sum())], BF,
                           kind="ExternalInput")
    w_d = nc.dram_tensor("wt", [128, KC, F], BF, kind="ExternalInput")
    negds_d = nc.dram_tensor("negds", [128, KC], FP, kind="ExternalInput")
    biasrow_d = nc.dram_tensor("biasrow", [1, F], BF, kind="ExternalInput")
    iota_d = nc.dram_tensor("iota", [128, 128], BF, kind="ExternalInput")
    ident_d = nc.dram_tensor("ident", [128, 128], BF, kind="ExternalInput")
    zeros_d = nc.dram_tensor("zeros", [128, 1], FP, kind="ExternalInput")
    out_d = nc.dram_tensor("out", [cfg.PAD_ROWS, F], BF,
                           kind="ExternalOutput")

    with tile.TileContext(nc) as tc:
        with (
            tc.tile_pool(name="const", bufs=1) as cpool,
            tc.tile_pool(name="gather", bufs=cfg.gather_bufs) as gpool,
            tc.tile_pool(name="amat", bufs=cfg.amat_bufs) as apool,
            tc.tile_pool(name="meta", bufs=8) as mpool,
            tc.tile_pool(name="work", bufs=4) as wpool,
            tc.tile_pool(name="pse1", bufs=2, space="PSUM") as e1pool,
            tc.tile_pool(name="pstr", bufs=2, space="PSUM") as trpool,
            tc.tile_pool(name="psout", bufs=2, space="PSUM") as opool,
        ):
            w_t = cpool.tile([128, KC, F], BF)
            negds_t = cpool.tile([128, KC], FP)
            biasrow_t = cpool.tile([1, F], BF)
            iota_t = cpool.tile([128, 128], BF)
            ident_t = cpool.tile([128, 128], BF)
            zeros_t = cpool.tile([128, 1], FP)
            ones_t = cpool.tile([1, 128], BF)
            nc.sync.dma_start(w_t[:], w_d[:])
            nc.sync.dma_start(negds_t[:], negds_d[:])
            nc.sync.dma_start(biasrow_t[:], biasrow_d[:])
            nc.sync.dma_start(iota_t[:], iota_d[:])
            nc.sync.dma_start(ident_t[:], ident_d[:])
            nc.sync.dma_start(zeros_t[:], zeros_d[:])
            nc.vector.memset(ones_t[:], 1.0)

            iota_rep = cpool.tile([128, C_MAXT, 128], BF)
            nc.vector.tensor_copy(
                iota_rep[:], iota_t[:, None, :].broadcast_to((128, C_MAXT, 128)))

            for stt in range(NST):
                t0, t1 = stt * ST, min((stt + 1) * ST, TILES)

                xgs = {}
                for bb in range(NB):
                    cg = int(C_stb[stt, bb])
                    if cg == 0:
                        continue
                    o16 = int(gcall_off[stt, bb]) * 8
                    idx_t = mpool.tile([128, 8 * C_MAXG], mybir.dt.int16,
                                       tag="idx")
                    # only partitions [32b, 32b+32) are read by queue b's
                    # Q7 pair; skip the other 3 replicas
                    nc.sync.dma_start(idx_t[32 * bb:32 * bb + 32, :8 * cg],
                                      idx_d[32 * bb:32 * bb + 32,
                                            o16:o16 + 8 * cg])
                    xg = gpool.tile([128, C_MAXG, F], BF, tag="xg")
                    nc.gpsimd.dma_gather(
                        xg[:, :cg, :],
                        xsrc[bb * cfg.BLK:(bb + 1) * cfg.BLK, :],
                        idx_t[:, :8 * cg],
                        num_idxs=128 * cg,
                        num_idxs_reg=128 * cg,
                        elem_size=F,
                        single_packet=False,
                        queue_num=bb,
                    )
                    xgs[bb] = xg

                for tt in range(t0, t1):
                    nt = int(n_t[tt])
                    toff = int(tile_coff[tt])

                    dest_t = mpool.tile([128, C_MAXT], BF, tag="dest")
                    nc.sync.dma_start(dest_t[:, :nt],
                                      dest_d[:, toff:toff + nt])
                    val_t = mpool.tile([128, C_MAXT], BF, tag="val")
                    nc.sync.dma_start(val_t[:, :nt], val_d[:, toff:toff + nt])
                    xot = wpool.tile([128, KC, 128], BF, tag="xot")
                    nc.sync.dma_start(xot[:], xot_d[:, tt])

                    # one-hot A: A[e, c, d] = (iota == dest[e,c]) * val[e,c]
                    a_t = apool.tile([128, C_MAXT, 128], BF, tag="a")
                    dest_b = dest_t[:, :nt, None].broadcast_to((128, nt, 128))
                    val_b = val_t[:, :nt, None].broadcast_to((128, nt, 128))
                    if cfg.flat_a:
                        a_f = a_t.rearrange("p a b -> p (a b)")[:, :nt * 128]
                        iota_f = iota_rep.rearrange(
                            "p a b -> p (a b)")[:, :nt * 128]
                        nc.vector.tensor_tensor(a_f, iota_f, dest_b,
                                                op=mybir.AluOpType.is_equal)
                        nc.vector.tensor_tensor(a_f, a_f, val_b,
                                                op=mybir.AluOpType.mult)
                    else:
                        nc.vector.tensor_tensor(a_t[:, :nt, :],
                                                iota_rep[:, :nt, :], dest_b,
                                                op=mybir.AluOpType.is_equal)
                        nc.vector.tensor_tensor(a_t[:, :nt, :],
                                                a_t[:, :nt, :], val_b,
                                                op=mybir.AluOpType.mult)

                    # segment-sum into PSUM
                    e1 = e1pool.tile([128, F], FP, tag="e1")
                    cc = 0
                    for bb in range(NB):
                        cb = int(C[cell_rank[tt, bb]])
                        if cb == 0:
                            continue
                        base = int((pad_off[cell_rank[tt, bb]] // 128)
                                   - gcall_off[stt, bb])
                        xg = xgs[bb]
                        for c in range(cb):
                            nc.tensor.matmul(
                                e1[:], a_t[:, cc, :], xg[:, base + c, :],
                                start=(cc == 0), stop=(cc == nt - 1),
                            )
                            cc += 1

                    e1_sb = wpool.tile([128, F], BF, tag="e1sb")
                    nc.scalar.activation(e1_sb[:], e1[:],
                                         mybir.ActivationFunctionType.Identity,
                                         bias=zeros_t[:])
                    e1T = trpool.tile([128, KC, 128], BF, tag="tr")
                    for kc in range(KC):
                        nc.tensor.transpose(e1T[:, kc, :],
                                            e1_sb[:, kc * 128:(kc + 1) * 128],
                                            ident_t[:])

                    e4T = wpool.tile([128, KC, 128], BF, tag="e4T")
                    for kc in range(KC):
                        nc.vector.scalar_tensor_tensor(
                            e4T[:, kc, :], e1T[:, kc, :],
                            negds_t[:, kc:kc + 1], xot[:, kc, :],
                            op0=mybir.AluOpType.mult,
                            op1=mybir.AluOpType.add)

                    ps_out = opool.tile([128, F], FP, tag="po")
                    for kc in range(KC):
                        nc.tensor.matmul(ps_out[:], e4T[:, kc, :],
                                         w_t[:, kc, :],
                                         start=(kc == 0), stop=False)
                    nc.tensor.matmul(ps_out[:], ones_t[:], biasrow_t[:],
                                     start=False, stop=True)
                    outs = wpool.tile([128, F], BF, tag="outs")
                    nc.scalar.activation(outs[:], ps_out[:],
                                         mybir.ActivationFunctionType.Identity,
                                         bias=zeros_t[:])
                    nc.sync.dma_start(out_d[tt * 128:(tt + 1) * 128, :],
                                      outs[:])

    nc.compile()
    return nc


def _make_in_maps(cfg, x, weight, learnable_diag, bias, row_perm,
                  idx_packed, dest_cols, val_cols):
    F, NC, TILES = cfg.F, cfg.NC, cfg.TILES
    KC = F // 128
    x16 = x.astype(BF_NP)
    w_host = np.ascontiguousarray(
        weight.reshape(KC, 128, F).transpose(1, 0, 2)).astype(BF_NP)
    negds_host = np.ascontiguousarray(
        -(learnable_diag + 1.0).reshape(KC, 128).T).astype(np.float32)
    biasrow_host = bias.reshape(1, F).astype(BF_NP)
    iota_host = np.tile(np.arange(128, dtype=np.float32)[None, :],
                        (128, 1)).astype(BF_NP)
    ident_host = np.eye(128, dtype=np.float32).astype(BF_NP)
    zeros_host = np.zeros((128, 1), dtype=np.float32)

    # residual x rows in device order (permuted), transposed per tile:
    # xot[p, t, kc, d] = xperm[t*128 + d, kc*128 + p]
    xperm = np.zeros((NC * cfg.PAD_ROWS, F), dtype=np.float32)
    valid = row_perm >= 0
    xperm[valid] = x[row_perm[valid]]
    xot_host = np.ascontiguousarray(
        xperm.reshape(NC, TILES, 128, KC, 128).transpose(0, 4, 1, 3, 2)
    ).astype(BF_NP)

    in_maps = []
    for c in range(NC):
        in_maps.append({
            "xsrc": x16,
            "xot": xot_host[c],
            "idx": np.ascontiguousarray(idx_packed[c]),
            "dest": np.ascontiguousarray(dest_cols[c]),
            "val": np.ascontiguousarray(val_cols[c]),
            "wt": w_host,
            "negds": negds_host,
            "biasrow": biasrow_host,
            "iota": iota_host,
            "ident": ident_host,
            "zeros": zeros_host,
        })
    return in_maps


def run(cfg, x, edge_row, edge_col, edge_val, weight, learnable_diag, bias,
        trace_dir=None):
    x = np.ascontiguousarray(np.asarray(x, dtype=np.float32))
    edge_row = np.asarray(edge_row).astype(np.int64)
    edge_col = np.asarray(edge_col).astype(np.int64)
    weight = np.asarray(weight, dtype=np.float32)
    learnable_diag = np.asarray(learnable_diag, dtype=np.float32)
    bias = np.asarray(bias, dtype=np.float32)

    row_perm, row_slot = _assign_rows(cfg, edge_row, edge_col)
    tables, idx_packed, dest_cols, val_cols = _preprocess(
        cfg, edge_row, edge_col, edge_val, row_slot)
    nc = _build(cfg, tables)
    in_maps = _make_in_maps(cfg, x, weight, learnable_diag, bias, row_perm,
                            idx_packed, dest_cols, val_cols)

    kwargs = {}
    if trace_dir:
        kwargs = dict(trace=True, tmpdir=trace_dir)
    res = run_bass_kernel_spmd(nc, in_maps, core_ids=list(range(cfg.NC)),
                               **kwargs)
    out = np.empty((cfg.N, cfg.F), dtype=np.float32)
    for c in range(cfg.NC):
        dev_rows = res.results[c]["out"].astype(np.float32)
        rp = row_perm[c * cfg.PAD_ROWS:(c + 1) * cfg.PAD_ROWS]
        m = rp >= 0
        out[rp[m]] = dev_rows[m]
    return out, res


def kernel(x, edge_row, edge_col, edge_val, weight, learnable_diag, bias,
           _want_trace=None):
    cfg = Cfg()
    out, res = run(cfg, x, edge_row, edge_col, edge_val, weight,
                   learnable_diag, bias, trace_dir=_want_trace)
    kernel._last_results = res
    return out
